# revision 23
# baseline (speedup 1.0000x reference)
"""Pairwise squared euclidean distances ||x_i - y_j||^2 on 8 NeuronCores.

Strategy: shard rows of x across cores (1024 rows each), replicate y.
Each core computes its natural [1024, 8192] tile d[m, n] = ||x_m - y_n||^2:
  - host precomputes (-2x)^T shard [128, 1024] fp16, y^T [128, 8192] fp16,
    y_sq replicated across partitions [128, 8192] fp16, x_sq per-partition
    columns [128, 8] f32;
  - PE: psum[m=128, n=1024] = xt_block.T @ yt_chunk (two K=128 fp16
    matmuls, f32 PSUM);
  - elementwise  out = (psum + x_sq[m]) + y_sq[n]  load-balanced across
    THREE engines (PSUM evacuation only on DVE/ACT; GpSimd has no PSUM
    port, so it gets pure-SBUF fp16 second passes):
      * type D  : DVE scalar_tensor_tensor, one pass per sub-block;
      * type AG : ACT activation(psum + x_sq bias) per sub-block, then one
                  GpSimd fp16 tensor_tensor (+y_sq) over the whole otile;
      * type AD : same but the fp16 +y_sq pass runs on DVE (2x perf mode);
  - output stored as fp16 [1024, 8192] (tolerance 2e-2 >> fp16's ~2.4e-4)
    which HALVES the dominant HBM store traffic; host upcasts to f32.
Startup: critical preloads split across the sync+scalar HWDGE rings
(fast first-byte); bulk y^T / y_sq ride gpsimd's SWDGE queue and are not
needed until ~20us in thanks to the n-outer / m-inner loop order.  The
first otile stores in 256KB halves to get the store stream flowing early;
steady-state stores are 32 x 512KB on the sync ring.
"""

import sys

sys.path.insert(0, "/opt/trn_rl_repo")

import numpy as np

import concourse.bass as bass
import concourse.mybir as mybir
import concourse.tile as tile
from concourse import bacc
from concourse.bass_utils import run_bass_kernel_spmd


def _ensure_axon_hooks_stub():
    """The agent image ships antenv without axon_hooks; bass_utils imports
    it when tracing is requested (e.g. BASS_TRACE=1 in the environment).
    Install a stub so that path degrades to no-trace instead of crashing."""
    try:
        import antenv.axon_hooks  # noqa: F401
        return
    except ImportError:
        pass
    import types
    try:
        import antenv
    except ImportError:
        return
    mod = types.ModuleType("antenv.axon_hooks")
    holder = {"hook": None}
    mod.set_axon_ntff_profile_hook = lambda h: holder.__setitem__("hook", h)
    mod.get_axon_ntff_profile_hook = lambda: holder["hook"]
    sys.modules["antenv.axon_hooks"] = mod
    antenv.axon_hooks = mod


_ensure_axon_hooks_stub()

N_CORES = 8
N, M, D = 8192, 8192, 128
R = N // N_CORES   # 1024 x-rows per core
P = 128            # SBUF partitions == D == m-block
NB = 512           # matmul moving block
OT = 2048          # n-cols per output tile / store (512KB fp16)
F32 = mybir.dt.float32
F16 = mybir.dt.float16

# per-SUB-BLOCK engine assignment:
#   D  = DVE one-pass scalar_tensor_tensor (1x rate; PSUM-capable);
#   AG = ACT evacuates PSUM (+x_sq bias), then a fp16 +y_sq tensor_tensor
#        on GpSimd.
# Key constraint discovered by measurement: concurrent GpSimd SBUF
# activity disables the DVE 2x perf mode, so DVE must run NO 2x ops
# (its STT is 1x anyway) for the GpSimd offload to pay off.  ACT has no
# exec queue (ENG_EXEC_QUEUE_DEPTH=0), costing ~300ns dead dispatch per
# op, so it gets the smaller share.  41 D : 23 AG balances
# DVE ~57us / GpSimd ~56us / ACT ~37us busy.
_AG_POS = {2, 5, 8, 11, 14, 16, 19, 22, 25, 28, 30, 33, 36, 39, 42,
           44, 47, 50, 53, 56, 58, 61, 63}
_SCHED = ["AG" if i in _AG_POS else "D" for i in range(64)]

_cached_nc = None


def _build():
    nc = bacc.Bacc("TRN2", target_bir_lowering=False, debug=False)

    xt_d = nc.dram_tensor("xt", [P, R], F16, kind="ExternalInput")       # (-2x)^T shard
    yt_d = nc.dram_tensor("yt", [P, M], F16, kind="ExternalInput")       # y^T
    ysr_d = nc.dram_tensor("ysr", [P, M], F16, kind="ExternalInput")     # y_sq replicated
    xsc_d = nc.dram_tensor("xsc", [P, R // P], F32, kind="ExternalInput")  # x_sq cols
    out_d = nc.dram_tensor("out", [R, M], F16, kind="ExternalOutput")    # natural tile
    xt, yt, ysr, xsc, out = (t.ap() for t in (xt_d, yt_d, ysr_d, xsc_d, out_d))

    add = mybir.AluOpType.add
    ident = mybir.ActivationFunctionType.Identity

    with tile.TileContext(nc) as tc:
        with (
            tc.tile_pool(name="persist", bufs=1) as persist,
            tc.tile_pool(name="outp", bufs=6) as outp,
            tc.tile_pool(name="ps", bufs=4, space=bass.MemorySpace.PSUM) as psp,
        ):
            xt_t = persist.tile([P, R], F16, tag="xt")
            xsc_t = persist.tile([P, R // P], F32, tag="xsc")
            yt_t = persist.tile([P, M], F16, tag="yt")
            ysr_t = persist.tile([P, M], F16, tag="ysr")

            # Critical-path preloads split across BOTH HWDGE rings so the
            # first matmul + STT inputs land ~2us after the preamble.
            nc.sync.dma_start(out=xt_t[:], in_=xt[:])
            nc.scalar.dma_start(out=xsc_t[:], in_=xsc[:])
            nc.sync.dma_start(out=yt_t[:, 0:1024], in_=yt[:, 0:1024])
            nc.scalar.dma_start(out=ysr_t[:, 0:1024], in_=ysr[:, 0:1024])
            nc.sync.dma_start(out=yt_t[:, 1024:OT], in_=yt[:, 1024:OT])
            nc.scalar.dma_start(out=ysr_t[:, 1024:OT], in_=ysr[:, 1024:OT])
            # Bulk loads also on the scalar ring (gpsimd now runs steady-
            # state compute; its SWDGE path is unused).  These have no
            # deps, so they dispatch ahead of the ACTIVATEs; cols 2048+
            # are first touched ~12us into each otile pass.
            nc.scalar.dma_start(out=yt_t[:, OT:3 * OT], in_=yt[:, OT:3 * OT])
            nc.scalar.dma_start(out=ysr_t[:, OT:3 * OT], in_=ysr[:, OT:3 * OT])
            nc.scalar.dma_start(out=yt_t[:, 3 * OT:M], in_=yt[:, 3 * OT:M])
            nc.scalar.dma_start(out=ysr_t[:, 3 * OT:M], in_=ysr[:, 3 * OT:M])

            oti = 0
            sbi = 0
            for ot_i in range(M // OT):      # 4 output-column tiles
                for mb in range(R // P):     # 8 m-blocks
                    o_t = outp.tile([P, OT], F16, tag="o")
                    xcol = xsc_t[:, mb:mb + 1]
                    for sb in range(OT // 1024):  # 2 sub-blocks
                        kind = _SCHED[sbi % len(_SCHED)]
                        sbi += 1
                        n0 = ot_i * OT + sb * 1024
                        os_ = slice(sb * 1024, (sb + 1) * 1024)
                        pt = psp.tile([P, 1024], F32, tag="pt")  # 2 PSUM banks
                        for ms in range(1024 // NB):
                            nc.tensor.matmul(
                                pt[:, ms * NB:(ms + 1) * NB],
                                xt_t[:, mb * P:(mb + 1) * P],
                                yt_t[:, n0 + ms * NB:n0 + (ms + 1) * NB],
                                start=True,
                                stop=True,
                            )
                        if kind == "D":
                            nc.vector.scalar_tensor_tensor(
                                out=o_t[:, os_],
                                in0=pt[:],
                                scalar=xcol,
                                in1=ysr_t[:, n0:n0 + 1024],
                                op0=add,
                                op1=add,
                            )
                        else:
                            nc.scalar.activation(
                                out=o_t[:, os_],
                                in_=pt[:],
                                func=ident,
                                bias=xcol,
                            )
                            nc.gpsimd.tensor_tensor(
                                out=o_t[:, os_],
                                in0=o_t[:, os_],
                                in1=ysr_t[:, n0:n0 + 1024],
                                op=add,
                            )
                    orows = out[mb * P:(mb + 1) * P, ot_i * OT:(ot_i + 1) * OT]
                    if oti == 0 or oti == 31:
                        # halves: gets the store stream flowing earlier at
                        # the head, and shortens the drain at the tail.
                        nc.sync.dma_start(out=orows[:, 0:1024], in_=o_t[:, 0:1024])
                        nc.sync.dma_start(out=orows[:, 1024:OT], in_=o_t[:, 1024:OT])
                    else:
                        nc.sync.dma_start(out=orows, in_=o_t[:])
                    oti += 1

    nc.compile()
    return nc


def _get_nc():
    global _cached_nc
    if _cached_nc is None:
        _cached_nc = _build()
    return _cached_nc


def _prep(x, y):
    x = np.asarray(x, dtype=np.float32)
    y = np.asarray(y, dtype=np.float32)
    yt16 = np.ascontiguousarray(y.T).astype(np.float16)
    ysq = np.sum(y.astype(np.float64) ** 2, axis=1).astype(np.float32)
    ysr16 = np.ascontiguousarray(
        np.broadcast_to(ysq.astype(np.float16)[None, :], (P, M)))
    xsqg = np.sum(x.astype(np.float64) ** 2, axis=1).astype(np.float32)
    xt_full = np.ascontiguousarray((-2.0 * x).T)  # [128, 8192]
    in_maps = []
    for c in range(N_CORES):
        rs = slice(c * R, (c + 1) * R)
        in_maps.append({
            "xt": np.ascontiguousarray(xt_full[:, rs]).astype(np.float16),
            "yt": yt16,
            "ysr": ysr16,
            "xsc": np.ascontiguousarray(xsqg[rs].reshape(R // P, P).T),
        })
    return in_maps


def run_raw(x, y, **kwargs):
    """Run the bass kernel; returns (full_output, BassKernelResults)."""
    in_maps = _prep(x, y)
    rr = run_bass_kernel_spmd(_get_nc(), in_maps, list(range(N_CORES)), **kwargs)
    full = np.empty((N, M), dtype=np.float32)
    for c in range(N_CORES):
        full[c * R:(c + 1) * R, :] = rr.results[c]["out"].astype(np.float32)
    return full, rr


def kernel(x, y):
    full, _ = run_raw(x, y)
    return full


# revision 27
# speedup vs baseline: 1.0386x; 1.0386x over previous
"""Pairwise squared euclidean distances ||x_i - y_j||^2 on 8 NeuronCores.

Strategy: shard rows of x across cores (1024 rows each), replicate y.
Each core computes its natural [1024, 8192] tile d[m, n] = ||x_m - y_n||^2:
  - host precomputes (-2x)^T shard [128, 1024] fp16, y^T [128, 8192] fp16,
    y_sq replicated across partitions [128, 8192] fp16, x_sq per-partition
    columns [128, 8] f32;
  - PE: psum[m=128, n=1024] = xt_block.T @ yt_chunk (two K=128 fp16
    matmuls, f32 PSUM);
  - elementwise  out = (psum + x_sq[m]) + y_sq[n]  load-balanced across
    THREE engines (PSUM evacuation only on DVE/ACT; GpSimd has no PSUM
    port, so it gets pure-SBUF fp16 second passes):
      * type D  : DVE scalar_tensor_tensor, one pass per sub-block;
      * type AG : ACT activation(psum + x_sq bias) per sub-block, then one
                  GpSimd fp16 tensor_tensor (+y_sq) over the whole otile;
      * type AD : same but the fp16 +y_sq pass runs on DVE (2x perf mode);
  - output stored as fp16 [1024, 8192] (tolerance 2e-2 >> fp16's ~2.4e-4)
    which HALVES the dominant HBM store traffic; host upcasts to f32.
Startup: critical preloads split across the sync+scalar HWDGE rings
(fast first-byte); bulk y^T / y_sq ride gpsimd's SWDGE queue and are not
needed until ~20us in thanks to the n-outer / m-inner loop order.  The
first otile stores in 256KB halves to get the store stream flowing early;
steady-state stores are 32 x 512KB on the sync ring.
"""

import sys

sys.path.insert(0, "/opt/trn_rl_repo")

import numpy as np

import concourse.bass as bass
import concourse.mybir as mybir
import concourse.tile as tile
from concourse import bacc
from concourse.bass_utils import run_bass_kernel_spmd


def _ensure_axon_hooks_stub():
    """The agent image ships antenv without axon_hooks; bass_utils imports
    it when tracing is requested (e.g. BASS_TRACE=1 in the environment).
    Install a stub so that path degrades to no-trace instead of crashing."""
    try:
        import antenv.axon_hooks  # noqa: F401
        return
    except ImportError:
        pass
    import types
    try:
        import antenv
    except ImportError:
        return
    mod = types.ModuleType("antenv.axon_hooks")
    holder = {"hook": None}
    mod.set_axon_ntff_profile_hook = lambda h: holder.__setitem__("hook", h)
    mod.get_axon_ntff_profile_hook = lambda: holder["hook"]
    sys.modules["antenv.axon_hooks"] = mod
    antenv.axon_hooks = mod


_ensure_axon_hooks_stub()

N_CORES = 8
N, M, D = 8192, 8192, 128
R = N // N_CORES   # 1024 x-rows per core
P = 128            # SBUF partitions == D == m-block
NB = 512           # matmul moving block
OT = 2048          # n-cols per output tile / store (512KB fp16)
F32 = mybir.dt.float32
F16 = mybir.dt.float16

# per-SUB-BLOCK engine assignment: D = DVE one-pass STT; AD = ACT
# evacuates PSUM (+x_sq bias), then a 1024-wide fp16 +y_sq tensor_tensor
# on DVE (2x perf mode engages at 1024 width only).  Measured dead ends:
# GpSimd steady-state compute poisons ALL concurrent DVE SBUF ops
# (~+20%, v5/v10); 2048-wide ops stall on sem latency with the thinner
# PSUM pipeline (v7); K=1 bias-matmul ysq pre-adds stall ACT on longer
# MM groups (v9); ratio shifts off 1:3 regressed (v8).  The simple 1:3
# split with fine-grained 1024 ops and 4-deep PSUM measures fastest.
_SCHED = ["D", "AD", "AD", "AD"] * 16

_cached_nc = None


def _build():
    nc = bacc.Bacc("TRN2", target_bir_lowering=False, debug=False)

    xt_d = nc.dram_tensor("xt", [P, R], F16, kind="ExternalInput")       # (-2x)^T shard
    yt_d = nc.dram_tensor("yt", [P, M], F16, kind="ExternalInput")       # y^T
    ysr_d = nc.dram_tensor("ysr", [P, M], F16, kind="ExternalInput")     # y_sq replicated
    xsc_d = nc.dram_tensor("xsc", [P, R // P], F32, kind="ExternalInput")  # x_sq cols
    out_d = nc.dram_tensor("out", [R, M], F16, kind="ExternalOutput")    # natural tile
    xt, yt, ysr, xsc, out = (t.ap() for t in (xt_d, yt_d, ysr_d, xsc_d, out_d))

    add = mybir.AluOpType.add
    ident = mybir.ActivationFunctionType.Identity

    with tile.TileContext(nc) as tc:
        with (
            tc.tile_pool(name="persist", bufs=1) as persist,
            tc.tile_pool(name="outp", bufs=8) as outp,
            tc.tile_pool(name="ps", bufs=4, space=bass.MemorySpace.PSUM) as psp,
        ):
            xt_t = persist.tile([P, R], F16, tag="xt")
            xsc_t = persist.tile([P, R // P], F32, tag="xsc")
            yt_t = persist.tile([P, M], F16, tag="yt")
            ysr_t = persist.tile([P, M], F16, tag="ysr")

            # Critical-path preloads split across BOTH HWDGE rings so the
            # first matmul + STT inputs land ~1us after the preamble.  The
            # first matmul only needs xt cols 0:128 and yt cols 0:512, so
            # those tiny pieces go first.
            nc.sync.dma_start(out=xt_t[:, 0:P], in_=xt[:, 0:P])
            nc.scalar.dma_start(out=xsc_t[:], in_=xsc[:])
            nc.sync.dma_start(out=yt_t[:, 0:1024], in_=yt[:, 0:1024])
            nc.scalar.dma_start(out=ysr_t[:, 0:1024], in_=ysr[:, 0:1024])
            nc.sync.dma_start(out=xt_t[:, P:R], in_=xt[:, P:R])
            nc.sync.dma_start(out=yt_t[:, 1024:OT], in_=yt[:, 1024:OT])
            nc.scalar.dma_start(out=ysr_t[:, 1024:OT], in_=ysr[:, 1024:OT])
            # Bulk on gpsimd (SWDGE, ~1MB chunks): issue-only work for the
            # otherwise-idle Q7; cols 2048+ first touched ~12us in.
            nc.gpsimd.dma_start(out=yt_t[:, OT:3 * OT], in_=yt[:, OT:3 * OT])
            nc.gpsimd.dma_start(out=ysr_t[:, OT:3 * OT], in_=ysr[:, OT:3 * OT])
            nc.gpsimd.dma_start(out=yt_t[:, 3 * OT:M], in_=yt[:, 3 * OT:M])
            nc.gpsimd.dma_start(out=ysr_t[:, 3 * OT:M], in_=ysr[:, 3 * OT:M])

            oti = 0
            sbi = 0
            for ot_i in range(M // OT):      # 4 output-column tiles
                for mb in range(R // P):     # 8 m-blocks
                    o_t = outp.tile([P, OT], F16, tag="o")
                    xcol = xsc_t[:, mb:mb + 1]
                    for sb in range(OT // 1024):  # 2 sub-blocks
                        kind = _SCHED[sbi % len(_SCHED)]
                        sbi += 1
                        n0 = ot_i * OT + sb * 1024
                        os_ = slice(sb * 1024, (sb + 1) * 1024)
                        pt = psp.tile([P, 1024], F32, tag="pt")  # 2 PSUM banks
                        for ms in range(1024 // NB):
                            nc.tensor.matmul(
                                pt[:, ms * NB:(ms + 1) * NB],
                                xt_t[:, mb * P:(mb + 1) * P],
                                yt_t[:, n0 + ms * NB:n0 + (ms + 1) * NB],
                                start=True,
                                stop=True,
                            )
                        if kind == "D":
                            nc.vector.scalar_tensor_tensor(
                                out=o_t[:, os_],
                                in0=pt[:],
                                scalar=xcol,
                                in1=ysr_t[:, n0:n0 + 1024],
                                op0=add,
                                op1=add,
                            )
                        else:
                            nc.scalar.activation(
                                out=o_t[:, os_],
                                in_=pt[:],
                                func=ident,
                                bias=xcol,
                            )
                            nc.vector.tensor_tensor(
                                out=o_t[:, os_],
                                in0=o_t[:, os_],
                                in1=ysr_t[:, n0:n0 + 1024],
                                op=add,
                            )
                    orows = out[mb * P:(mb + 1) * P, ot_i * OT:(ot_i + 1) * OT]
                    if oti == 0 or oti == 31:
                        # halves: gets the store stream flowing earlier at
                        # the head, and shortens the drain at the tail.
                        nc.sync.dma_start(out=orows[:, 0:1024], in_=o_t[:, 0:1024])
                        nc.sync.dma_start(out=orows[:, 1024:OT], in_=o_t[:, 1024:OT])
                    else:
                        nc.sync.dma_start(out=orows, in_=o_t[:])
                    oti += 1

    nc.compile()
    return nc


def _get_nc():
    global _cached_nc
    if _cached_nc is None:
        _cached_nc = _build()
    return _cached_nc


def _prep(x, y):
    x = np.asarray(x, dtype=np.float32)
    y = np.asarray(y, dtype=np.float32)
    yt16 = np.ascontiguousarray(y.T).astype(np.float16)
    ysq = np.sum(y.astype(np.float64) ** 2, axis=1).astype(np.float32)
    ysr16 = np.ascontiguousarray(
        np.broadcast_to(ysq.astype(np.float16)[None, :], (P, M)))
    xsqg = np.sum(x.astype(np.float64) ** 2, axis=1).astype(np.float32)
    xt_full = np.ascontiguousarray((-2.0 * x).T)  # [128, 8192]
    in_maps = []
    for c in range(N_CORES):
        rs = slice(c * R, (c + 1) * R)
        in_maps.append({
            "xt": np.ascontiguousarray(xt_full[:, rs]).astype(np.float16),
            "yt": yt16,
            "ysr": ysr16,
            "xsc": np.ascontiguousarray(xsqg[rs].reshape(R // P, P).T),
        })
    return in_maps


def run_raw(x, y, **kwargs):
    """Run the bass kernel; returns (full_output, BassKernelResults)."""
    in_maps = _prep(x, y)
    rr = run_bass_kernel_spmd(_get_nc(), in_maps, list(range(N_CORES)), **kwargs)
    full = np.empty((N, M), dtype=np.float32)
    for c in range(N_CORES):
        full[c * R:(c + 1) * R, :] = rr.results[c]["out"].astype(np.float32)
    return full, rr


def kernel(x, y):
    full, _ = run_raw(x, y)
    return full


# revision 30
# speedup vs baseline: 1.1204x; 1.0788x over previous
"""Pairwise squared euclidean distances ||x_i - y_j||^2 on 8 NeuronCores.

Strategy: shard rows of x across cores (1024 rows each), replicate y.
Each core computes its natural [1024, 8192] tile d[m, n] = ||x_m - y_n||^2:
  - host precomputes (-2x)^T shard [128, 1024] fp16, y^T [128, 8192] fp16,
    y_sq replicated across partitions [128, 8192] fp16, x_sq per-partition
    columns [128, 8] f32;
  - PE: psum[m=128, n=1024] = xt_block.T @ yt_chunk (two K=128 fp16
    matmuls, f32 PSUM);
  - elementwise  out = (psum + x_sq[m]) + y_sq[n]  load-balanced across
    THREE engines (PSUM evacuation only on DVE/ACT; GpSimd has no PSUM
    port, so it gets pure-SBUF fp16 second passes):
      * type D  : DVE scalar_tensor_tensor, one pass per sub-block;
      * type AG : ACT activation(psum + x_sq bias) per sub-block, then one
                  GpSimd fp16 tensor_tensor (+y_sq) over the whole otile;
      * type AD : same but the fp16 +y_sq pass runs on DVE (2x perf mode);
  - output stored as fp16 [1024, 8192] (tolerance 2e-2 >> fp16's ~2.4e-4)
    which HALVES the dominant HBM store traffic; host upcasts to f32.
Startup: critical preloads split across the sync+scalar HWDGE rings
(fast first-byte); bulk y^T / y_sq ride gpsimd's SWDGE queue and are not
needed until ~20us in thanks to the n-outer / m-inner loop order.  The
first otile stores in 256KB halves to get the store stream flowing early;
steady-state stores are 32 x 512KB on the sync ring.
"""

import sys

sys.path.insert(0, "/opt/trn_rl_repo")

import numpy as np

import concourse.bass as bass
import concourse.mybir as mybir
import concourse.tile as tile
from concourse import bacc
from concourse.bass_utils import run_bass_kernel_spmd


def _ensure_axon_hooks_stub():
    """The agent image ships antenv without axon_hooks; bass_utils imports
    it when tracing is requested (e.g. BASS_TRACE=1 in the environment).
    Install a stub so that path degrades to no-trace instead of crashing."""
    try:
        import antenv.axon_hooks  # noqa: F401
        return
    except ImportError:
        pass
    import types
    try:
        import antenv
    except ImportError:
        return
    mod = types.ModuleType("antenv.axon_hooks")
    holder = {"hook": None}
    mod.set_axon_ntff_profile_hook = lambda h: holder.__setitem__("hook", h)
    mod.get_axon_ntff_profile_hook = lambda: holder["hook"]
    sys.modules["antenv.axon_hooks"] = mod
    antenv.axon_hooks = mod


_ensure_axon_hooks_stub()

N_CORES = 8
N, M, D = 8192, 8192, 128
R = N // N_CORES   # 1024 x-rows per core
P = 128            # SBUF partitions == D == m-block
NB = 512           # matmul moving block
OT = 2048          # n-cols per output tile / store (512KB fp16)
F32 = mybir.dt.float32
F16 = mybir.dt.float16

# per-SUB-BLOCK engine assignment: D = DVE one-pass STT; AD = ACT
# evacuates PSUM (+x_sq bias), then a 1024-wide fp16 +y_sq tensor_tensor
# on DVE (2x perf mode engages at 1024 width only).  Measured dead ends:
# GpSimd steady-state compute poisons ALL concurrent DVE SBUF ops
# (~+20%, v5/v10); 2048-wide ops stall on sem latency with the thinner
# PSUM pipeline (v7); K=1 bias-matmul ysq pre-adds stall ACT on longer
# MM groups (v9); ratio shifts off 1:3 regressed (v8).  The simple 1:3
# split with fine-grained 1024 ops and 4-deep PSUM measures fastest.
_SCHED = ["D", "AD", "AD", "AD"] * 16

_cached_nc = None


def _build():
    nc = bacc.Bacc("TRN2", target_bir_lowering=False, debug=False)

    xt_d = nc.dram_tensor("xt", [P, R], F16, kind="ExternalInput")       # (-2x)^T shard
    yt_d = nc.dram_tensor("yt", [P, M], F16, kind="ExternalInput")       # y^T
    ysr_d = nc.dram_tensor("ysr", [P, M], F16, kind="ExternalInput")     # y_sq replicated
    xsc_d = nc.dram_tensor("xsc", [P, R // P], F32, kind="ExternalInput")  # x_sq cols
    out_d = nc.dram_tensor("out", [R, M], F16, kind="ExternalOutput")    # natural tile
    xt, yt, ysr, xsc, out = (t.ap() for t in (xt_d, yt_d, ysr_d, xsc_d, out_d))

    add = mybir.AluOpType.add
    ident = mybir.ActivationFunctionType.Identity

    with tile.TileContext(nc) as tc:
        with (
            tc.tile_pool(name="persist", bufs=1) as persist,
            tc.tile_pool(name="outp", bufs=6) as outp,
            tc.tile_pool(name="ps", bufs=4, space=bass.MemorySpace.PSUM) as psp,
        ):
            xt_t = persist.tile([P, R], F16, tag="xt")
            xsc_t = persist.tile([P, R // P], F32, tag="xsc")
            yt_t = persist.tile([P, M], F16, tag="yt")
            ysr_t = persist.tile([P, M], F16, tag="ysr")

            # Critical-path preloads split across BOTH HWDGE rings so the
            # first matmul + STT inputs land ~2us after the preamble.
            nc.sync.dma_start(out=xt_t[:], in_=xt[:])
            nc.scalar.dma_start(out=xsc_t[:], in_=xsc[:])
            nc.sync.dma_start(out=yt_t[:, 0:1024], in_=yt[:, 0:1024])
            nc.scalar.dma_start(out=ysr_t[:, 0:1024], in_=ysr[:, 0:1024])
            nc.sync.dma_start(out=yt_t[:, 1024:OT], in_=yt[:, 1024:OT])
            nc.scalar.dma_start(out=ysr_t[:, 1024:OT], in_=ysr[:, 1024:OT])
            # Bulk on gpsimd (SWDGE, ~1MB chunks): issue-only work for the
            # otherwise-idle Q7; cols 2048+ first touched ~12us in.
            nc.gpsimd.dma_start(out=yt_t[:, OT:3 * OT], in_=yt[:, OT:3 * OT])
            nc.gpsimd.dma_start(out=ysr_t[:, OT:3 * OT], in_=ysr[:, OT:3 * OT])
            nc.gpsimd.dma_start(out=yt_t[:, 3 * OT:M], in_=yt[:, 3 * OT:M])
            nc.gpsimd.dma_start(out=ysr_t[:, 3 * OT:M], in_=ysr[:, 3 * OT:M])

            oti = 0
            sbi = 0
            for ot_i in range(M // OT):      # 4 output-column tiles
                for mb in range(R // P):     # 8 m-blocks
                    o_t = outp.tile([P, OT], F16, tag="o")
                    xcol = xsc_t[:, mb:mb + 1]
                    for sb in range(OT // 1024):  # 2 sub-blocks
                        kind = _SCHED[sbi % len(_SCHED)]
                        sbi += 1
                        n0 = ot_i * OT + sb * 1024
                        os_ = slice(sb * 1024, (sb + 1) * 1024)
                        pt = psp.tile([P, 1024], F32, tag="pt")  # 2 PSUM banks
                        for ms in range(1024 // NB):
                            nc.tensor.matmul(
                                pt[:, ms * NB:(ms + 1) * NB],
                                xt_t[:, mb * P:(mb + 1) * P],
                                yt_t[:, n0 + ms * NB:n0 + (ms + 1) * NB],
                                start=True,
                                stop=True,
                            )
                        if kind == "D":
                            nc.vector.scalar_tensor_tensor(
                                out=o_t[:, os_],
                                in0=pt[:],
                                scalar=xcol,
                                in1=ysr_t[:, n0:n0 + 1024],
                                op0=add,
                                op1=add,
                            )
                        else:
                            nc.scalar.activation(
                                out=o_t[:, os_],
                                in_=pt[:],
                                func=ident,
                                bias=xcol,
                            )
                            nc.vector.tensor_tensor(
                                out=o_t[:, os_],
                                in0=o_t[:, os_],
                                in1=ysr_t[:, n0:n0 + 1024],
                                op=add,
                            )
                    orows = out[mb * P:(mb + 1) * P, ot_i * OT:(ot_i + 1) * OT]
                    if oti == 0:
                        # halves: get the store stream flowing earlier
                        nc.sync.dma_start(out=orows[:, 0:1024], in_=o_t[:, 0:1024])
                        nc.sync.dma_start(out=orows[:, 1024:OT], in_=o_t[:, 1024:OT])
                    else:
                        nc.sync.dma_start(out=orows, in_=o_t[:])
                    oti += 1

    nc.compile()
    return nc


def _get_nc():
    global _cached_nc
    if _cached_nc is None:
        _cached_nc = _build()
    return _cached_nc


def _prep(x, y):
    x = np.asarray(x, dtype=np.float32)
    y = np.asarray(y, dtype=np.float32)
    yt16 = np.ascontiguousarray(y.T).astype(np.float16)
    ysq = np.sum(y.astype(np.float64) ** 2, axis=1).astype(np.float32)
    ysr16 = np.ascontiguousarray(
        np.broadcast_to(ysq.astype(np.float16)[None, :], (P, M)))
    xsqg = np.sum(x.astype(np.float64) ** 2, axis=1).astype(np.float32)
    xt_full = np.ascontiguousarray((-2.0 * x).T)  # [128, 8192]
    in_maps = []
    for c in range(N_CORES):
        rs = slice(c * R, (c + 1) * R)
        in_maps.append({
            "xt": np.ascontiguousarray(xt_full[:, rs]).astype(np.float16),
            "yt": yt16,
            "ysr": ysr16,
            "xsc": np.ascontiguousarray(xsqg[rs].reshape(R // P, P).T),
        })
    return in_maps


def run_raw(x, y, **kwargs):
    """Run the bass kernel; returns (full_output, BassKernelResults)."""
    in_maps = _prep(x, y)
    rr = run_bass_kernel_spmd(_get_nc(), in_maps, list(range(N_CORES)), **kwargs)
    full = np.empty((N, M), dtype=np.float32)
    for c in range(N_CORES):
        full[c * R:(c + 1) * R, :] = rr.results[c]["out"].astype(np.float32)
    return full, rr


def kernel(x, y):
    full, _ = run_raw(x, y)
    return full
